# revision 35
# baseline (speedup 1.0000x reference)
"""Multi-head attention (B=4, S=2048, D=1024, H=16) on 8 TRN2 NeuronCores.

Sharding: core c <- batch c//2, heads 8*(c%2) .. 8*(c%2)+8 (Megatron-style:
Wq/Wk/Wv column-parallel, Wo row-parallel). No collectives: the two partial
outputs per batch are summed on the host (plus the bo bias).

Per-core kernel strategy (all matmul operands fp16; host pre-converts):
  - The scalar engine's 256 exp ACTIVATEs ([128,1024] each, ~294us total)
    are the hard floor; everything else is arranged to hide under them.
  - Stage 2 keeps the full PE array busy so HAM stays at K=8/8:
    scores = two concurrent row-group matmuls (both heads of a pair),
    ctx and the softmax denominator = concurrent col-tiled pairs (M=64),
    with the denominator from a ones[128,64] stationary, which lands l
    replicated across 64 partitions (full-width reciprocal, no broadcast).
  - v is projected first (ctx needs it almost immediately); q/k for
    head-pair 0 follow; the remaining q/k projections and most of the
    output projection are fed into stage 2's tensor slack through a
    background-work generator, with explicit add_dep_helper edges at the
    DVE-write -> matmul-stationary-read hazards Tile misses.
"""
import itertools
import sys

sys.path.insert(0, "/opt/trn_rl_repo")
import numpy as np

import concourse.bass as bass
import concourse.bacc as bacc
import concourse.mybir as mybir
import concourse.tile as tile
from concourse.tile import add_dep_helper
from concourse.bass_utils import run_bass_kernel_spmd

f32 = mybir.dt.float32
f16 = mybir.dt.float16
EXP = mybir.ActivationFunctionType.Exp

S = 2048          # sequence length
D = 1024          # model dim
HC = 8            # heads per core
DK = 64           # head dim
JC = HC * DK      # per-core projection width (512)
SCALE = 0.125     # 1/sqrt(DK)
N_CORES = 8


class _NS:
    pass


def build_nc():
    nc = bacc.Bacc(None, target_bir_lowering=False, debug=False)

    io = _NS()
    io.qt = nc.dram_tensor("qt", [D, S], f16, kind="ExternalInput")
    io.kt = nc.dram_tensor("kt", [D, S], f16, kind="ExternalInput")
    io.vt = nc.dram_tensor("vt", [D, S], f16, kind="ExternalInput")
    io.wqt = nc.dram_tensor("wqt", [D, JC], f16, kind="ExternalInput")
    io.wkt = nc.dram_tensor("wkt", [D, JC], f16, kind="ExternalInput")
    io.wvt = nc.dram_tensor("wvt", [D, JC], f16, kind="ExternalInput")
    io.wot = nc.dram_tensor("wot", [JC, D], f16, kind="ExternalInput")
    io.bq = nc.dram_tensor("bq", [128, 4], f32, kind="ExternalInput")
    io.bk = nc.dram_tensor("bk", [128, 4], f32, kind="ExternalInput")
    io.bvb = nc.dram_tensor("bvb", [128, JC], f32, kind="ExternalInput")
    io.out = nc.dram_tensor("out", [S, D], f16, kind="ExternalOutput")

    with tile.TileContext(nc) as tc:
        with (
            tc.tile_pool(name="big", bufs=1) as big,
            tc.tile_pool(name="work", bufs=3) as work,
            tc.tile_pool(name="xpool", bufs=1) as xp,
            tc.tile_pool(name="xvpool", bufs=4) as xvp,
            tc.tile_pool(name="att", bufs=8) as att,
            tc.tile_pool(name="att2", bufs=2) as att2,
        ):
            sb = _NS()
            sb.qT_sb = big.tile([128, 4, S], f16)           # [p, jt, s]
            sb.kT_sb = big.tile([128, 4, S], f16)
            sb.v_sb = big.tile([128, 16, HC, DK], f16)      # [p, st, h, c]
            sb.ones_sb = big.tile([128, DK], f16)
            sb.wq_sb = big.tile([128, 8, JC], f16)
            sb.wk_sb = big.tile([128, 8, JC], f16)
            sb.wv_sb = big.tile([128, 8, JC], f16)
            sb.bq_sb = big.tile([128, 4], f32)
            sb.bk_sb = big.tile([128, 4], f32)
            sb.bvb_sb = big.tile([128, JC], f32)
            sb.ctxn_sb = big.tile([128, 4, S], f16)         # [p, pair, s]
            sb.wot_sb = big.tile([128, 4, D], f16)

            nc.vector.memset(sb.ones_sb[:], 1.0)

            bias_insts = {}   # (key, jt, sc) -> bias-add instruction
            mul_insts = {}    # (sqb, pair, half) -> [mul instructions]

            def _dep(reader, writer, why):
                # Tile misses DVE-write -> matmul-stationary-read deps when
                # emission is tightly interleaved; add the edge explicitly.
                f = reader.ins if isinstance(reader, bass.BassInstruction) else reader
                t = writer.ins if isinstance(writer, bass.BassInstruction) else writer
                add_dep_helper(f, t, sync=True, reason=why)

            xq = [None, None]
            xk = [None, None]

            def dma_x(x_dram, tag, sc, ts):
                t = xp.tile([128, 8, 1024], f16, tag=f"{tag}{sc}")
                nc.sync.dma_start(
                    t[:],
                    x_dram[:, sc * 1024:(sc + 1) * 1024].rearrange(
                        "(kt p) s -> p kt s", p=128
                    ),
                )
                ts[sc] = t

            def qk_fill(pool, tag, key, x_ts, w_sb, o_sb, b_sb, jt, sc,
                        halves=(0, 1)):
                """[128,512] projection half-fills: 1-bank background slots
                that double-buffer."""
                for hf in halves:
                    ps = pool.tile([128, 512], f32, tag=tag,
                                   name=f"pj_{key}_{jt}_{sc}_{hf}")
                    s0 = hf * 512
                    for kt in range(8):
                        w = w_sb[:, kt, jt * 128:(jt + 1) * 128]
                        nc.tensor.matmul(ps[:], w, x_ts[sc][:, kt, s0:s0 + 512],
                                         start=(kt == 0), stop=(kt == 7))
                        if kt % 2:
                            yield
                    bias_insts[(key, jt, sc, hf)] = nc.vector.tensor_scalar_add(
                        o_sb[:, jt, sc * 1024 + s0:sc * 1024 + s0 + 512],
                        ps[:], b_sb[:, jt:jt + 1])
                    yield

            def run(gen):
                for _ in gen:
                    pass

            v_bias = {}

            def v_fill_step(pool, st):
                for _ in v_fill_gen(pool, st):
                    pass

            def v_fill_gen(pool, st, tag="projv"):
                xv = xvp.tile([128, 8, 128], f16, tag="xv")
                nc.sync.dma_start(
                    xv[:],
                    io.vt[:, st * 128:(st + 1) * 128].rearrange(
                        "(kt p) s -> p kt s", p=128
                    ),
                )
                ps = pool.tile([128, JC], f32, tag=tag,
                               name=f"vps_{st}")
                for kt in range(8):
                    nc.tensor.matmul(ps[:], xv[:, kt, :], sb.wv_sb[:, kt, :],
                                     start=(kt == 0), stop=(kt == 7))
                    if kt % 2:
                        yield
                v_bias[st] = nc.vector.tensor_add(
                    sb.v_sb[:, st, :, :],
                    ps[:].rearrange("p (h c) -> p h c", h=HC),
                    sb.bvb_sb[:].rearrange("p (h c) -> p h c", h=HC),
                )
                yield

            # --- upfront: v first; the q/k x/w DMA issues are interleaved
            # into the xv DMA sequence so the Sync queue (which is in-order
            # and paced by the xv slot rotation) doesn't delay them.
            nc.sync.dma_start(sb.wv_sb[:], io.wvt.rearrange("(kt p) j -> p kt j", p=128))
            nc.sync.dma_start(sb.bvb_sb[:], io.bvb[:])
            fatv_cm = tc.tile_pool(name="fatv", bufs=2, space="PSUM")
            fatv = fatv_cm.__enter__()
            with (
                tc.tile_pool(name="fat", bufs=2, space="PSUM") as fat,
            ):
                for st in range(8):
                    v_fill_step(fatv, st)
                    if st == 1:
                        nc.sync.dma_start(
                            sb.wq_sb[:],
                            io.wqt.rearrange("(kt p) j -> p kt j", p=128))
                        nc.sync.dma_start(sb.bq_sb[:], io.bq[:])
                        dma_x(io.qt, "xq", 0, xq)
                    if st == 3:
                        nc.sync.dma_start(
                            sb.wk_sb[:],
                            io.wkt.rearrange("(kt p) j -> p kt j", p=128))
                        nc.sync.dma_start(sb.bk_sb[:], io.bk[:])
                        dma_x(io.kt, "xk", 0, xk)
                    if st == 5:
                        dma_x(io.kt, "xk", 1, xk)
                run(qk_fill(fat, "proj", "q", xq, sb.wq_sb, sb.qT_sb, sb.bq_sb, 0, 0))
                run(qk_fill(fat, "proj", "k", xk, sb.wk_sb, sb.kT_sb, sb.bk_sb, 0, 0))
                dma_x(io.qt, "xq", 1, xq)
                nc.sync.dma_start(
                    sb.wot_sb[:], io.wot.rearrange("(kt p) j -> p kt j", p=128))

            # --- stage 2/3 with background stage-1 work -------------------
            with (
                tc.tile_pool(name="ps2st", bufs=2, space="PSUM") as pp_st,
                tc.tile_pool(name="ps2cl", bufs=1, space="PSUM") as pp_cl,
            ):
                pp_pj = fatv
                def stage3_chunk(sq2, tagit):
                    sqb_r, half_r = sq2 // 8, (sq2 % 8) // 4
                    for n in range(2):
                        ps = pp_pj.tile([128, 512], f32, tag=next(tagit),
                                        name=f"o_{sq2}_{n}")
                        for p in range(4):
                            omm = nc.tensor.matmul(
                                ps[:],
                                sb.ctxn_sb[:, p, sq2 * 128:(sq2 + 1) * 128],
                                sb.wot_sb[:, p, n * 512:(n + 1) * 512],
                                start=(p == 0), stop=(p == 3),
                            )
                            if n == 0:
                                for m in mul_insts[(sqb_r, p, half_r)]:
                                    _dep(omm, m, f"out({sq2}) after ctxn")
                            if p % 2:
                                yield
                        ob = work.tile([128, 512], f16, tag="ob")
                        nc.vector.tensor_copy(ob[:], ps[:, 0:512])
                        nc.sync.dma_start(
                            io.out[sq2 * 128:(sq2 + 1) * 128,
                                   n * 512:(n + 1) * 512],
                            ob[:],
                        )
                        yield

                def bg_qk():
                    # ordered by deadline: k-jt0-c1 by group 8, v st8-15 by
                    # ~group 2k, pair p (group 32p) needs q-sc0/k-sc0/k-sc1
                    # of jt=p; the q-sc1 fills are only read in s_q block 1.
                    q_args = ("q", xq, sb.wq_sb, sb.qT_sb, sb.bq_sb)
                    k_args = ("k", xk, sb.wk_sb, sb.kT_sb, sb.bk_sb)
                    yield from qk_fill(pp_pj, "projv", "k", xk, sb.wk_sb,
                                       sb.kT_sb, sb.bk_sb, 0, 1)
                    for st in range(8, 16):
                        yield from v_fill_gen(pp_pj, st)
                    for jt in range(1, 4):
                        for (key, x_ts, w_sb, o_sb, b_sb), sc in (
                            (q_args, 0), (k_args, 0), (k_args, 1),
                        ):
                            yield from qk_fill(pp_pj, "projv", key, x_ts, w_sb,
                                               o_sb, b_sb, jt, sc)
                    for jt in range(0, 4):
                        key, x_ts, w_sb, o_sb, b_sb = q_args
                        yield from qk_fill(pp_pj, "projv", key, x_ts, w_sb,
                                           o_sb, b_sb, jt, 1)

                def bg_s3():
                    # output projection for s_q block 0 (runs during block 1)
                    tagit = itertools.cycle(["projv"])
                    for sq2 in range(8):
                        yield from stage3_chunk(sq2, tagit)

                def bg_s3b():
                    # block-1 rows whose ctxn (half 0) is already complete
                    tagit = itertools.cycle(["projv"])
                    for sq2 in range(8, 12):
                        yield from stage3_chunk(sq2, tagit)

                bgs = [bg_qk()]

                def pump(n=1):
                    done = 0
                    while bgs and done < n:
                        try:
                            next(bgs[0])
                            done += 1
                        except StopIteration:
                            bgs.pop(0)

                state = {}

                def emit_cl(g, pt):
                    sqb, pair, half, k = g
                    ctx, lx = state[(sqb, pair, half)]
                    h0, h1 = 2 * pair, 2 * pair + 1
                    st0, sp0 = (k == 0), (k == 15)
                    if (sqb, pair, half) == (0, 0, 0):
                        # ensure the v projection for this k-tile is emitted,
                        # then guard the stationary read explicitly.
                        while k not in v_bias and bgs:
                            pump(1)
                    cmm = nc.tensor.matmul(ctx[0:64, :], sb.v_sb[:, k, h0, :],
                                     pt[:, 0:512], start=st0, stop=sp0,
                                     skip_group_check=True)
                    if (sqb, pair, half) == (0, 0, 0):
                        _dep(cmm, v_bias[k], f"ctx(k={k}) after v bias")
                    nc.tensor.matmul(ctx[64:128, :], sb.v_sb[:, k, h1, :],
                                     pt[:, 512:1024], start=st0, stop=sp0,
                                     skip_group_check=True)
                    nc.tensor.matmul(lx[0:64, :], sb.ones_sb[:],
                                     pt[:, 0:512], start=st0, stop=sp0,
                                     skip_group_check=True)
                    nc.tensor.matmul(lx[64:128, :], sb.ones_sb[:],
                                     pt[:, 512:1024], start=st0, stop=sp0,
                                     skip_group_check=True)

                def normalize(g):
                    sqb, pair, half, _ = g
                    ctx, lx = state.pop((sqb, pair, half))
                    sq0 = sqb * 1024 + half * 512
                    # evacuate psum first so the ctx/l banks free up for the
                    # next half's matmuls; l is already partition-replicated.
                    cc = att2.tile([128, 512], f32, tag="cc",
                                   name=f"cc_{sqb}_{pair}_{half}")
                    nc.vector.tensor_copy(cc[:], ctx[:])
                    lc = att2.tile([128, 512], f32, tag="lc",
                                   name=f"lc_{sqb}_{pair}_{half}")
                    nc.vector.tensor_copy(lc[:], lx[:])
                    r = att2.tile([128, 512], f32, tag="r",
                                  name=f"r_{sqb}_{pair}_{half}")
                    nc.vector.reciprocal_approx_fast(r[:], lc[:])
                    mul_insts[(sqb, pair, half)] = [
                        nc.vector.tensor_mul(
                            sb.ctxn_sb[0:64, pair, sq0:sq0 + 512],
                            cc[0:64, :], r[0:64, :],
                        ),
                        nc.vector.tensor_mul(
                            sb.ctxn_sb[64:128, pair, sq0:sq0 + 512],
                            cc[64:128, :], r[64:128, :],
                        ),
                    ]

                groups = [(sqb, pair, half, k)
                          for sqb in range(2) for pair in range(4)
                          for half in range(2) for k in range(16)]
                # ctx/l trail the scores/exp stream by 2 groups so a ctx
                # matmul waiting on the single cl psum slot at a half
                # boundary has two score-groups queued ahead of it (the
                # tensor queue is in-order; a stalled ctx MM would
                # otherwise delay the next scores and gap the ACT).
                pending = []
                for g in groups:
                    sqb, pair, half, k = g
                    if k == 0:
                        if (sqb, pair, half) == (1, 0, 0):
                            bgs.append(bg_s3())
                        state[(sqb, pair, half)] = (
                            pp_cl.tile([128, 512], f32, tag="ctx",
                                       name=f"ctx_{sqb}_{pair}_{half}"),
                            pp_cl.tile([128, 512], f32, tag="l",
                                       name=f"l_{sqb}_{pair}_{half}"),
                        )
                    if (sqb, pair, half, k) == (1, 3, 1, 3):
                        bgs.append(bg_s3b())
                    sq0 = sqb * 1024 + half * 512
                    st = pp_st.tile([128, 1024], f32, tag="st")
                    smm = nc.tensor.matmul(
                        st[:, 0:512],
                        sb.kT_sb[0:64, pair, k * 128:(k + 1) * 128],
                        sb.qT_sb[0:64, pair, sq0:sq0 + 512],
                        start=True, stop=True,
                    )
                    why = f"scores({sqb},{pair}) after qk bias"
                    if k == 0 and not (sqb == 0 and pair == 0):
                        _dep(smm, bias_insts[("q", pair, sqb, half)], why)
                    if half == 0 and k % 4 == 0 and not (
                            sqb == 0 and pair == 0 and k == 0):
                        _dep(smm, bias_insts[("k", pair, k // 8, (k // 4) % 2)], why)
                    nc.tensor.matmul(
                        st[:, 512:1024],
                        sb.kT_sb[64:128, pair, k * 128:(k + 1) * 128],
                        sb.qT_sb[64:128, pair, sq0:sq0 + 512],
                        start=True, stop=True,
                    )
                    pt = att.tile([128, 1024], f16, tag="pt")
                    nc.scalar.activation(pt[:], st[:], EXP, scale=SCALE)
                    pending.append((g, pt))
                    if len(pending) > 4:
                        pg = pending.pop(0)
                        emit_cl(*pg)
                        if pg[0][3] == 15:
                            normalize(pg[0])
                    gi = groups.index(g)
                    pump(3 if gi < 16 else 2)
                for pg in pending:
                    emit_cl(*pg)
                    if pg[0][3] == 15:
                        normalize(pg[0])

                # drain any remaining background work
                while bgs:
                    try:
                        next(bgs[0])
                    except StopIteration:
                        bgs.pop(0)

                # final output rows (need the very last ctxn half)
                tagit = itertools.cycle(["projv", "ctx", "l"])
                for sq2 in range(12, 16):
                    for n in range(2):
                        tg = next(tagit)
                        ps = (pp_pj if tg == "projv" else pp_cl).tile(
                            [128, 512], f32, tag=tg, name=f"o_{sq2}_{n}")
                        for p in range(4):
                            omm = nc.tensor.matmul(
                                ps[:],
                                sb.ctxn_sb[:, p, sq2 * 128:(sq2 + 1) * 128],
                                sb.wot_sb[:, p, n * 512:(n + 1) * 512],
                                start=(p == 0), stop=(p == 3),
                            )
                            if n == 0:
                                for m in mul_insts[(sq2 // 8, p, (sq2 % 8) // 4)]:
                                    _dep(omm, m, f"out({sq2}) after ctxn")
                        ob = work.tile([128, 512], f16, tag="ob")
                        nc.vector.tensor_copy(ob[:], ps[:])
                        nc.sync.dma_start(
                            io.out[sq2 * 128:(sq2 + 1) * 128,
                                   n * 512:(n + 1) * 512],
                            ob[:],
                        )

            fatv_cm.__exit__(None, None, None)

    nc.compile()
    return nc


_NC = None


def _get_nc():
    global _NC
    if _NC is None:
        _NC = build_nc()
    return _NC


def make_in_maps(Q, K, V, Wq, bq, Wk, bk, Wv, bv, Wo, bo):
    ash = lambda x: np.ascontiguousarray(np.asarray(x, dtype=np.float32).astype(np.float16))
    asf = lambda x: np.ascontiguousarray(np.asarray(x, dtype=np.float32))
    in_maps = []
    for c in range(N_CORES):
        b = c // 2
        j0 = JC * (c % 2)
        jsl = slice(j0, j0 + JC)
        in_maps.append({
            "qt": ash(np.asarray(Q)[b].T),
            "kt": ash(np.asarray(K)[b].T),
            "vt": ash(np.asarray(V)[b].T),
            "wqt": ash(np.asarray(Wq)[jsl].T),
            "wkt": ash(np.asarray(Wk)[jsl].T),
            "wvt": ash(np.asarray(Wv)[jsl].T),
            "wot": ash(np.asarray(Wo)[:, jsl].T),
            "bq": asf(np.asarray(bq)[jsl].reshape(4, 128).T),
            "bk": asf(np.asarray(bk)[jsl].reshape(4, 128).T),
            "bvb": asf(np.broadcast_to(np.asarray(bv)[jsl], (128, JC))),
        })
    return in_maps


def kernel(Q, K, V, Wq, bq, Wk, bk, Wv, bv, Wo, bo, _trace=False, _trace_kwargs=None):
    nc = _get_nc()
    in_maps = make_in_maps(Q, K, V, Wq, bq, Wk, bk, Wv, bv, Wo, bo)
    res = run_bass_kernel_spmd(
        nc, in_maps, core_ids=list(range(N_CORES)),
        trace=_trace, **(_trace_kwargs or {}),
    )
    parts = [res.results[c]["out"].astype(np.float32) for c in range(N_CORES)]
    bo_np = np.asarray(bo, dtype=np.float32)
    O = np.stack([parts[2 * b] + parts[2 * b + 1] + bo_np for b in range(4)])
    kernel.last_results = res
    return O.astype(np.float32)


# revision 37
# speedup vs baseline: 1.0162x; 1.0162x over previous
"""Multi-head attention (B=4, S=2048, D=1024, H=16) on 8 TRN2 NeuronCores.

Sharding: core c <- batch c//2, heads 8*(c%2) .. 8*(c%2)+8 (Megatron-style:
Wq/Wk/Wv column-parallel, Wo row-parallel). No collectives: the two partial
outputs per batch are summed on the host (plus the bo bias).

Per-core kernel strategy (all matmul operands fp16; host pre-converts):
  - The scalar engine's 256 exp ACTIVATEs ([128,1024] each, ~294us total)
    are the hard floor; everything else is arranged to hide under them.
  - Stage 2 keeps the full PE array busy so HAM stays at K=8/8:
    scores = two concurrent row-group matmuls (both heads of a pair),
    ctx and the softmax denominator = concurrent col-tiled pairs (M=64),
    with the denominator from a ones[128,64] stationary, which lands l
    replicated across 64 partitions (full-width reciprocal, no broadcast).
  - v is projected first (ctx needs it almost immediately); q/k for
    head-pair 0 follow; the remaining q/k projections and most of the
    output projection are fed into stage 2's tensor slack through a
    background-work generator, with explicit add_dep_helper edges at the
    DVE-write -> matmul-stationary-read hazards Tile misses.
"""
import itertools
import sys

sys.path.insert(0, "/opt/trn_rl_repo")
import numpy as np

import concourse.bass as bass
import concourse.bacc as bacc
import concourse.mybir as mybir
import concourse.tile as tile
from concourse.tile import add_dep_helper
from concourse.bass_utils import run_bass_kernel_spmd

f32 = mybir.dt.float32
f16 = mybir.dt.float16
EXP = mybir.ActivationFunctionType.Exp

S = 2048          # sequence length
D = 1024          # model dim
HC = 8            # heads per core
DK = 64           # head dim
JC = HC * DK      # per-core projection width (512)
SCALE = 0.125     # 1/sqrt(DK)
N_CORES = 8


class _NS:
    pass


def build_nc():
    nc = bacc.Bacc(None, target_bir_lowering=False, debug=False)

    io = _NS()
    io.qt = nc.dram_tensor("qt", [D, S], f16, kind="ExternalInput")
    io.kt = nc.dram_tensor("kt", [D, S], f16, kind="ExternalInput")
    io.vt = nc.dram_tensor("vt", [D, S], f16, kind="ExternalInput")
    io.wqt = nc.dram_tensor("wqt", [D, JC], f16, kind="ExternalInput")
    io.wkt = nc.dram_tensor("wkt", [D, JC], f16, kind="ExternalInput")
    io.wvt = nc.dram_tensor("wvt", [D, JC], f16, kind="ExternalInput")
    io.wot = nc.dram_tensor("wot", [JC, D], f16, kind="ExternalInput")
    io.bq = nc.dram_tensor("bq", [128, 4], f32, kind="ExternalInput")
    io.bk = nc.dram_tensor("bk", [128, 4], f32, kind="ExternalInput")
    io.bvb = nc.dram_tensor("bvb", [128, JC], f32, kind="ExternalInput")
    io.out = nc.dram_tensor("out", [S, D], f16, kind="ExternalOutput")

    with tile.TileContext(nc) as tc:
        with (
            tc.tile_pool(name="big", bufs=1) as big,
            tc.tile_pool(name="work", bufs=3) as work,
            tc.tile_pool(name="xpool", bufs=1) as xp,
            tc.tile_pool(name="xvpool", bufs=4) as xvp,
            tc.tile_pool(name="att", bufs=7) as att,
            tc.tile_pool(name="att2", bufs=2) as att2,
        ):
            sb = _NS()
            sb.qT_sb = big.tile([128, 4, S], f16)           # [p, jt, s]
            sb.kT_sb = big.tile([128, 4, S], f16)
            sb.v_sb = big.tile([128, 16, HC, DK], f16)      # [p, st, h, c]
            sb.ones_sb = big.tile([128, DK], f16)
            sb.wq_sb = big.tile([128, 8, JC], f16)
            sb.wk_sb = big.tile([128, 8, JC], f16)
            sb.wv_sb = big.tile([128, 8, JC], f16)
            sb.bq_sb = big.tile([128, 4], f32)
            sb.bk_sb = big.tile([128, 4], f32)
            sb.bvb_sb = big.tile([128, JC], f32)
            sb.ctxn_sb = big.tile([128, 4, S], f16)         # [p, pair, s]
            sb.wot_sb = big.tile([128, 4, D], f16)

            nc.vector.memset(sb.ones_sb[:], 1.0)

            bias_insts = {}   # (key, jt, sc) -> bias-add instruction
            mul_insts = {}    # (sqb, pair, half) -> [mul instructions]

            def _dep(reader, writer, why):
                # Tile misses DVE-write -> matmul-stationary-read deps when
                # emission is tightly interleaved; add the edge explicitly.
                f = reader.ins if isinstance(reader, bass.BassInstruction) else reader
                t = writer.ins if isinstance(writer, bass.BassInstruction) else writer
                add_dep_helper(f, t, sync=True, reason=why)

            xq = [None, None]
            xk = [None, None]

            def dma_x(x_dram, tag, sc, ts):
                t = xp.tile([128, 8, 1024], f16, tag=f"{tag}{sc}")
                nc.sync.dma_start(
                    t[:],
                    x_dram[:, sc * 1024:(sc + 1) * 1024].rearrange(
                        "(kt p) s -> p kt s", p=128
                    ),
                )
                ts[sc] = t

            def qk_fill(pool, tag, key, x_ts, w_sb, o_sb, b_sb, jt, sc,
                        halves=(0, 1)):
                """[128,512] projection half-fills: 1-bank background slots
                that double-buffer."""
                for hf in halves:
                    ps = pool.tile([128, 512], f32, tag=tag,
                                   name=f"pj_{key}_{jt}_{sc}_{hf}")
                    s0 = hf * 512
                    for kt in range(8):
                        w = w_sb[:, kt, jt * 128:(jt + 1) * 128]
                        nc.tensor.matmul(ps[:], w, x_ts[sc][:, kt, s0:s0 + 512],
                                         start=(kt == 0), stop=(kt == 7))
                        if kt % 2:
                            yield
                    bias_insts[(key, jt, sc, hf)] = nc.vector.tensor_scalar_add(
                        o_sb[:, jt, sc * 1024 + s0:sc * 1024 + s0 + 512],
                        ps[:], b_sb[:, jt:jt + 1])
                    yield

            def run(gen):
                for _ in gen:
                    pass

            v_bias = {}

            def v_fill_step(pool, st):
                for _ in v_fill_gen(pool, st):
                    pass

            def v_fill_gen(pool, st, tag="projv"):
                xv = xvp.tile([128, 8, 128], f16, tag="xv")
                nc.sync.dma_start(
                    xv[:],
                    io.vt[:, st * 128:(st + 1) * 128].rearrange(
                        "(kt p) s -> p kt s", p=128
                    ),
                )
                ps = pool.tile([128, JC], f32, tag=tag,
                               name=f"vps_{st}")
                for kt in range(8):
                    nc.tensor.matmul(ps[:], xv[:, kt, :], sb.wv_sb[:, kt, :],
                                     start=(kt == 0), stop=(kt == 7))
                    if kt % 2:
                        yield
                v_bias[st] = nc.vector.tensor_add(
                    sb.v_sb[:, st, :, :],
                    ps[:].rearrange("p (h c) -> p h c", h=HC),
                    sb.bvb_sb[:].rearrange("p (h c) -> p h c", h=HC),
                )
                yield

            # --- upfront: v first; the q/k x/w DMA issues are interleaved
            # into the xv DMA sequence so the Sync queue (which is in-order
            # and paced by the xv slot rotation) doesn't delay them.
            nc.sync.dma_start(sb.wv_sb[:], io.wvt.rearrange("(kt p) j -> p kt j", p=128))
            nc.sync.dma_start(sb.bvb_sb[:], io.bvb[:])
            fatv_cm = tc.tile_pool(name="fatv", bufs=2, space="PSUM")
            fatv = fatv_cm.__enter__()
            with (
                tc.tile_pool(name="fat", bufs=2, space="PSUM") as fat,
            ):
                for st in range(8):
                    v_fill_step(fatv, st)
                    if st == 1:
                        nc.sync.dma_start(
                            sb.wq_sb[:],
                            io.wqt.rearrange("(kt p) j -> p kt j", p=128))
                        nc.sync.dma_start(sb.bq_sb[:], io.bq[:])
                        dma_x(io.qt, "xq", 0, xq)
                    if st == 3:
                        nc.sync.dma_start(
                            sb.wk_sb[:],
                            io.wkt.rearrange("(kt p) j -> p kt j", p=128))
                        nc.sync.dma_start(sb.bk_sb[:], io.bk[:])
                        dma_x(io.kt, "xk", 0, xk)
                    if st == 5:
                        dma_x(io.kt, "xk", 1, xk)
                run(qk_fill(fat, "proj", "q", xq, sb.wq_sb, sb.qT_sb, sb.bq_sb, 0, 0))
                run(qk_fill(fat, "proj", "k", xk, sb.wk_sb, sb.kT_sb, sb.bk_sb, 0, 0))

            # --- stage 2/3 with background stage-1 work -------------------
            with (
                tc.tile_pool(name="ps2st", bufs=2, space="PSUM") as pp_st,
                tc.tile_pool(name="ps2cl", bufs=1, space="PSUM") as pp_cl,
            ):
                pp_pj = fatv
                def stage3_chunk(sq2, tagit):
                    sqb_r, half_r = sq2 // 8, (sq2 % 8) // 4
                    for n in range(2):
                        ps = pp_pj.tile([128, 512], f32, tag=next(tagit),
                                        name=f"o_{sq2}_{n}")
                        for p in range(4):
                            omm = nc.tensor.matmul(
                                ps[:],
                                sb.ctxn_sb[:, p, sq2 * 128:(sq2 + 1) * 128],
                                sb.wot_sb[:, p, n * 512:(n + 1) * 512],
                                start=(p == 0), stop=(p == 3),
                            )
                            if n == 0:
                                for m in mul_insts[(sqb_r, p, half_r)]:
                                    _dep(omm, m, f"out({sq2}) after ctxn")
                            if p % 2:
                                yield
                        ob = work.tile([128, 512], f16, tag="ob")
                        nc.vector.tensor_copy(ob[:], ps[:, 0:512])
                        nc.sync.dma_start(
                            io.out[sq2 * 128:(sq2 + 1) * 128,
                                   n * 512:(n + 1) * 512],
                            ob[:],
                        )
                        yield

                def bg_qk():
                    # ordered by deadline: k-jt0-c1 by group 8, v st8-15 by
                    # ~group 2k, pair p (group 32p) needs q-sc0/k-sc0/k-sc1
                    # of jt=p; the q-sc1 fills are only read in s_q block 1.
                    q_args = ("q", xq, sb.wq_sb, sb.qT_sb, sb.bq_sb)
                    k_args = ("k", xk, sb.wk_sb, sb.kT_sb, sb.bk_sb)
                    yield from qk_fill(pp_pj, "projv", "k", xk, sb.wk_sb,
                                       sb.kT_sb, sb.bk_sb, 0, 1)
                    for st in range(8, 16):
                        yield from v_fill_gen(pp_pj, st)
                    for jt in range(1, 4):
                        for (key, x_ts, w_sb, o_sb, b_sb), sc in (
                            (q_args, 0), (k_args, 0), (k_args, 1),
                        ):
                            yield from qk_fill(pp_pj, "projv", key, x_ts, w_sb,
                                               o_sb, b_sb, jt, sc)
                        if jt == 1:
                            # 6MB that nothing reads before group ~62; keeping
                            # these out of the startup DMA queue lets the
                            # background xv transfers land on time.
                            dma_x(io.qt, "xq", 1, xq)
                            nc.sync.dma_start(
                                sb.wot_sb[:],
                                io.wot.rearrange("(kt p) j -> p kt j", p=128))
                    for jt in range(0, 4):
                        key, x_ts, w_sb, o_sb, b_sb = q_args
                        yield from qk_fill(pp_pj, "projv", key, x_ts, w_sb,
                                           o_sb, b_sb, jt, 1)

                def bg_s3():
                    # output projection for s_q block 0 (runs during block 1)
                    tagit = itertools.cycle(["projv"])
                    for sq2 in range(8):
                        yield from stage3_chunk(sq2, tagit)

                def bg_s3b():
                    # block-1 rows whose ctxn (half 0) is already complete
                    tagit = itertools.cycle(["projv"])
                    for sq2 in range(8, 12):
                        yield from stage3_chunk(sq2, tagit)

                bgs = [bg_qk()]

                def pump(n=1):
                    done = 0
                    while bgs and done < n:
                        try:
                            next(bgs[0])
                            done += 1
                        except StopIteration:
                            bgs.pop(0)

                state = {}

                def emit_cl(g, pt):
                    sqb, pair, half, k = g
                    ctx, lx = state[(sqb, pair, half)]
                    h0, h1 = 2 * pair, 2 * pair + 1
                    st0, sp0 = (k == 0), (k == 15)
                    if (sqb, pair, half) == (0, 0, 0):
                        # ensure the v projection for this k-tile is emitted,
                        # then guard the stationary read explicitly.
                        while k not in v_bias and bgs:
                            pump(1)
                    cmm = nc.tensor.matmul(ctx[0:64, :], sb.v_sb[:, k, h0, :],
                                     pt[:, 0:512], start=st0, stop=sp0,
                                     skip_group_check=True)
                    if (sqb, pair, half) == (0, 0, 0):
                        _dep(cmm, v_bias[k], f"ctx(k={k}) after v bias")
                    nc.tensor.matmul(ctx[64:128, :], sb.v_sb[:, k, h1, :],
                                     pt[:, 512:1024], start=st0, stop=sp0,
                                     skip_group_check=True)
                    nc.tensor.matmul(lx[0:64, :], sb.ones_sb[:],
                                     pt[:, 0:512], start=st0, stop=sp0,
                                     skip_group_check=True)
                    nc.tensor.matmul(lx[64:128, :], sb.ones_sb[:],
                                     pt[:, 512:1024], start=st0, stop=sp0,
                                     skip_group_check=True)

                def normalize(g):
                    sqb, pair, half, _ = g
                    ctx, lx = state.pop((sqb, pair, half))
                    sq0 = sqb * 1024 + half * 512
                    # evacuate psum first so the ctx/l banks free up for the
                    # next half's matmuls; l is already partition-replicated.
                    cc = att2.tile([128, 512], f32, tag="cc",
                                   name=f"cc_{sqb}_{pair}_{half}")
                    nc.vector.tensor_copy(cc[:], ctx[:])
                    lc = att2.tile([128, 512], f32, tag="lc",
                                   name=f"lc_{sqb}_{pair}_{half}")
                    nc.vector.tensor_copy(lc[:], lx[:])
                    r = att2.tile([128, 512], f32, tag="r",
                                  name=f"r_{sqb}_{pair}_{half}")
                    nc.vector.reciprocal_approx_fast(r[:], lc[:])
                    mul_insts[(sqb, pair, half)] = [
                        nc.vector.tensor_mul(
                            sb.ctxn_sb[0:64, pair, sq0:sq0 + 512],
                            cc[0:64, :], r[0:64, :],
                        ),
                        nc.vector.tensor_mul(
                            sb.ctxn_sb[64:128, pair, sq0:sq0 + 512],
                            cc[64:128, :], r[64:128, :],
                        ),
                    ]

                groups = [(sqb, pair, half, k)
                          for sqb in range(2) for pair in range(4)
                          for half in range(2) for k in range(16)]
                # ctx/l trail the scores/exp stream by 2 groups so a ctx
                # matmul waiting on the single cl psum slot at a half
                # boundary has two score-groups queued ahead of it (the
                # tensor queue is in-order; a stalled ctx MM would
                # otherwise delay the next scores and gap the ACT).
                pending = []
                for g in groups:
                    sqb, pair, half, k = g
                    if k == 0:
                        if (sqb, pair, half) == (1, 0, 0):
                            bgs.append(bg_s3())
                        state[(sqb, pair, half)] = (
                            pp_cl.tile([128, 512], f32, tag="ctx",
                                       name=f"ctx_{sqb}_{pair}_{half}"),
                            pp_cl.tile([128, 512], f32, tag="l",
                                       name=f"l_{sqb}_{pair}_{half}"),
                        )
                    if (sqb, pair, half, k) == (1, 3, 1, 3):
                        bgs.append(bg_s3b())
                    sq0 = sqb * 1024 + half * 512
                    st = pp_st.tile([128, 1024], f32, tag="st")
                    smm = nc.tensor.matmul(
                        st[:, 0:512],
                        sb.kT_sb[0:64, pair, k * 128:(k + 1) * 128],
                        sb.qT_sb[0:64, pair, sq0:sq0 + 512],
                        start=True, stop=True,
                    )
                    why = f"scores({sqb},{pair}) after qk bias"
                    if k == 0 and not (sqb == 0 and pair == 0):
                        _dep(smm, bias_insts[("q", pair, sqb, half)], why)
                    if half == 0 and k % 4 == 0 and not (
                            sqb == 0 and pair == 0 and k == 0):
                        _dep(smm, bias_insts[("k", pair, k // 8, (k // 4) % 2)], why)
                    nc.tensor.matmul(
                        st[:, 512:1024],
                        sb.kT_sb[64:128, pair, k * 128:(k + 1) * 128],
                        sb.qT_sb[64:128, pair, sq0:sq0 + 512],
                        start=True, stop=True,
                    )
                    pt = att.tile([128, 1024], f16, tag="pt")
                    nc.scalar.activation(pt[:], st[:], EXP, scale=SCALE)
                    pending.append((g, pt))
                    if len(pending) > 3:
                        pg = pending.pop(0)
                        emit_cl(*pg)
                        if pg[0][3] == 15:
                            normalize(pg[0])
                    gi = groups.index(g)
                    pump(3 if gi < 16 else 2)
                for pg in pending:
                    emit_cl(*pg)
                    if pg[0][3] == 15:
                        normalize(pg[0])

                # drain any remaining background work
                while bgs:
                    try:
                        next(bgs[0])
                    except StopIteration:
                        bgs.pop(0)

                # final output rows (need the very last ctxn half)
                tagit = itertools.cycle(["projv", "ctx", "l"])
                for sq2 in range(12, 16):
                    for n in range(2):
                        tg = next(tagit)
                        ps = (pp_pj if tg == "projv" else pp_cl).tile(
                            [128, 512], f32, tag=tg, name=f"o_{sq2}_{n}")
                        for p in range(4):
                            omm = nc.tensor.matmul(
                                ps[:],
                                sb.ctxn_sb[:, p, sq2 * 128:(sq2 + 1) * 128],
                                sb.wot_sb[:, p, n * 512:(n + 1) * 512],
                                start=(p == 0), stop=(p == 3),
                            )
                            if n == 0:
                                for m in mul_insts[(sq2 // 8, p, (sq2 % 8) // 4)]:
                                    _dep(omm, m, f"out({sq2}) after ctxn")
                        ob = work.tile([128, 512], f16, tag="ob")
                        nc.vector.tensor_copy(ob[:], ps[:])
                        nc.sync.dma_start(
                            io.out[sq2 * 128:(sq2 + 1) * 128,
                                   n * 512:(n + 1) * 512],
                            ob[:],
                        )

            fatv_cm.__exit__(None, None, None)

    nc.compile()
    return nc


_NC = None


def _get_nc():
    global _NC
    if _NC is None:
        _NC = build_nc()
    return _NC


def make_in_maps(Q, K, V, Wq, bq, Wk, bk, Wv, bv, Wo, bo):
    ash = lambda x: np.ascontiguousarray(np.asarray(x, dtype=np.float32).astype(np.float16))
    asf = lambda x: np.ascontiguousarray(np.asarray(x, dtype=np.float32))
    in_maps = []
    for c in range(N_CORES):
        b = c // 2
        j0 = JC * (c % 2)
        jsl = slice(j0, j0 + JC)
        in_maps.append({
            "qt": ash(np.asarray(Q)[b].T),
            "kt": ash(np.asarray(K)[b].T),
            "vt": ash(np.asarray(V)[b].T),
            "wqt": ash(np.asarray(Wq)[jsl].T),
            "wkt": ash(np.asarray(Wk)[jsl].T),
            "wvt": ash(np.asarray(Wv)[jsl].T),
            "wot": ash(np.asarray(Wo)[:, jsl].T),
            "bq": asf(np.asarray(bq)[jsl].reshape(4, 128).T),
            "bk": asf(np.asarray(bk)[jsl].reshape(4, 128).T),
            "bvb": asf(np.broadcast_to(np.asarray(bv)[jsl], (128, JC))),
        })
    return in_maps


def kernel(Q, K, V, Wq, bq, Wk, bk, Wv, bv, Wo, bo, _trace=False, _trace_kwargs=None):
    nc = _get_nc()
    in_maps = make_in_maps(Q, K, V, Wq, bq, Wk, bk, Wv, bv, Wo, bo)
    res = run_bass_kernel_spmd(
        nc, in_maps, core_ids=list(range(N_CORES)),
        trace=_trace, **(_trace_kwargs or {}),
    )
    parts = [res.results[c]["out"].astype(np.float32) for c in range(N_CORES)]
    bo_np = np.asarray(bo, dtype=np.float32)
    O = np.stack([parts[2 * b] + parts[2 * b + 1] + bo_np for b in range(4)])
    kernel.last_results = res
    return O.astype(np.float32)
